# revision 5
# baseline (speedup 1.0000x reference)
"""Multi-head attention block on 8 TRN2 NeuronCores.

Problem: x[2,2048,768] -> qkv proj -> 12-head attention -> out proj.
Sharding: 24 (batch, head) pairs across 8 cores; core c handles batch
c//4 and heads 3*(c%4)..3*(c%4)+2. Each core computes its heads'
Q,K,V, attention, and a partial output projection; the host sums the
four per-batch partials and adds the bias terms.

Device math notes:
  - All matmuls run in float32r (full-rate PE, ~tf32 precision).
  - Softmax: exp without max subtraction (logits ~ N(0,1), safe in f32);
    denominators come from a ones column appended to V; the division is
    a K=1 broadcast matmul of the reciprocal row + one DVE multiply.
  - k-bias is dropped (softmax is shift invariant along keys);
    v-bias and proj-bias are folded in on the host:
      out += b_proj + b_v @ w_proj   (softmax rows sum to 1).
  - q-bias and the 1/sqrt(D) scale are fused into the PSUM->SBUF copy
    of Q^T on the scalar engine.
"""

import os
import sys

for _p in ("/opt/trn_rl_repo", "/opt/pypackages"):
    if _p not in sys.path:
        sys.path.append(_p)

import numpy as np

B, N, C = 2, 2048, 768
H, D = 12, 64
HPC = 3                    # heads per core
J = HPC * D                # 192: per-core head-dim rows
NCORES = 8
NBLK = 512                 # query-block free size
NB = N // NBLK             # 4
MC = N // 128              # 16 key chunks
KC = C // 128              # 6 contraction chunks for projections

_cache = {}
LAST_RESULTS = None


def _build():
    import concourse.mybir as mybir
    import concourse.tile as tile
    from concourse import bacc

    f32 = mybir.dt.float32
    f32r = mybir.dt.float32r
    Exp = mybir.ActivationFunctionType.Exp
    Identity = mybir.ActivationFunctionType.Identity

    nc = bacc.Bacc("TRN2", target_bir_lowering=False, debug=False,
                   num_devices=NCORES)

    xt_d = nc.declare_dram_parameter("xt", [C, N], f32r, isOutput=False)
    wq_d = nc.declare_dram_parameter("wq", [C, J], f32r, isOutput=False)
    wk_d = nc.declare_dram_parameter("wk", [C, J], f32r, isOutput=False)
    wv_d = nc.declare_dram_parameter("wv", [C, J], f32r, isOutput=False)
    bq_d = nc.declare_dram_parameter("bq", [J, 1], f32, isOutput=False)
    ones_d = nc.declare_dram_parameter("ones", [128, 67], f32r,
                                       isOutput=False)
    wp_d = nc.declare_dram_parameter("wp", [J, C], f32r, isOutput=False)
    out_d = nc.declare_dram_parameter("out", [N, C], f32, isOutput=True)

    # J=192 rows live on two SBUF tiles: rows 0:128 and 128:192.
    MCH = [(0, 128), (128, 64)]

    with tile.TileContext(nc) as tc:
        with (
            tc.tile_pool(name="persist", bufs=1) as pp,
            tc.tile_pool(name="etile", bufs=4) as pe,
            tc.tile_pool(name="small", bufs=3) as psm,
            tc.tile_pool(name="osb", bufs=3) as posb,
        ):
            xt = [pp.tile([128, N], f32r, tag=f"xt{i}", name=f"xt{i}")
                  for i in range(KC)]
            wq = [pp.tile([128, J], f32r, tag=f"wq{i}", name=f"wq{i}")
                  for i in range(KC)]
            wk = [pp.tile([128, J], f32r, tag=f"wk{i}", name=f"wk{i}")
                  for i in range(KC)]
            wv = [pp.tile([128, J], f32r, tag=f"wv{i}", name=f"wv{i}")
                  for i in range(KC)]
            for i in range(KC):
                sl = slice(128 * i, 128 * (i + 1))
                nc.sync.dma_start(xt[i][:], xt_d[sl, :])
                nc.sync.dma_start(wq[i][:], wq_d[sl, :])
                nc.sync.dma_start(wk[i][:], wk_d[sl, :])
                nc.sync.dma_start(wv[i][:], wv_d[sl, :])
            wpa = pp.tile([128, C], f32r, tag="wpa", name="wpa")
            wpb = pp.tile([64, C], f32r, tag="wpb", name="wpb")
            nc.sync.dma_start(wpa[:], wp_d[0:128, :])
            nc.sync.dma_start(wpb[:], wp_d[128:192, :])
            bqa = pp.tile([128, 1], f32, tag="bqa", name="bqa")
            bqb = pp.tile([64, 1], f32, tag="bqb", name="bqb")
            nc.sync.dma_start(bqa[:], bq_d[0:128, :])
            nc.sync.dma_start(bqb[:], bq_d[128:192, :])

            qa = pp.tile([128, N], f32r, tag="qa", name="qa")
            qb = pp.tile([64, N], f32r, tag="qb", name="qb")
            ka = pp.tile([128, N], f32r, tag="ka", name="ka")
            kb = pp.tile([64, N], f32r, tag="kb", name="kb")
            aa = pp.tile([128, N], f32r, tag="aa", name="aa")
            ab = pp.tile([64, N], f32r, tag="ab", name="ab")
            # V with a ones column per head: [128, 3*65]
            vx = [pp.tile([128, HPC * 65], f32r, tag=f"vx{m}", name=f"vx{m}")
                  for m in range(MC)]
            ones_t = pp.tile([128, 67], f32r, tag="ones_t", name="ones_t")
            nc.sync.dma_start(ones_t[:], ones_d[:, :])
            ones64 = ones_t[0:1, 0:64]
            for m in range(MC):
                on = vx[m].rearrange("p (h e) -> p h e", e=65)[:, :, 64:65]
                nc.vector.tensor_copy(
                    on, ones_t[:, 64:67].rearrange("p (h e) -> p h e", e=1))

            # ---- Phase 1: Q^T, K^T (d-major) and V (token-major) ----
            with tc.tile_pool(name="ps1", bufs=2, space="PSUM") as ps1:
                for t, (r0, rn) in enumerate(MCH):
                    qdst, kdst = (qa, ka) if t == 0 else (qb, kb)
                    bq_t = bqa if t == 0 else bqb
                    for nb in range(NB):
                        nsl = slice(NBLK * nb, NBLK * (nb + 1))
                        ps = ps1.tile([128, NBLK], f32, tag="qk", name="ps_q")
                        for k in range(KC):
                            nc.tensor.matmul(
                                ps[:rn, :], wq[k][:, r0:r0 + rn],
                                xt[k][:, nsl],
                                start=(k == 0), stop=(k == KC - 1))
                        nc.scalar.activation(
                            qdst[:rn, nsl], ps[:rn, :], Identity,
                            bias=bq_t[:rn, :], scale=0.125)
                        ps = ps1.tile([128, NBLK], f32, tag="qk", name="ps_k")
                        for k in range(KC):
                            nc.tensor.matmul(
                                ps[:rn, :], wk[k][:, r0:r0 + rn],
                                xt[k][:, nsl],
                                start=(k == 0), stop=(k == KC - 1))
                        nc.vector.tensor_copy(kdst[:rn, nsl], ps[:rn, :])
                for m in range(MC):
                    msl = slice(128 * m, 128 * (m + 1))
                    ps = ps1.tile([128, J], f32, tag="v", name="ps_v")
                    for k in range(KC):
                        nc.tensor.matmul(ps[:], xt[k][:, msl], wv[k][:],
                                         start=(k == 0), stop=(k == KC - 1))
                    vdst = vx[m].rearrange("p (h e) -> p h e", e=65)[:, :, 0:64]
                    nc.vector.tensor_copy(
                        vdst, ps.rearrange("p (h e) -> p h e", e=64))

            # ---- Phase 2: attention per head / query block ----
            with tc.tile_pool(name="ps2", bufs=1, space="PSUM") as ps2:
                for h in range(HPC):
                    if h < 2:
                        qsrc, ksrc, adst, r0 = qa, ka, aa, 64 * h
                    else:
                        qsrc, ksrc, adst, r0 = qb, kb, ab, 0
                    rr = slice(r0, r0 + 64)
                    for nb in range(NB):
                        nsl = slice(NBLK * nb, NBLK * (nb + 1))
                        av = ps2.tile([65, NBLK], f32, tag="av", bufs=2,
                                      name="ps_av")
                        pend = []
                        for m in range(MC):
                            msl = slice(128 * m, 128 * (m + 1))
                            s = ps2.tile([128, NBLK], f32, tag="s", bufs=3,
                                         name="ps_s")
                            nc.tensor.matmul(s[:], ksrc[rr, msl],
                                             qsrc[rr, nsl])
                            e = pe.tile([128, NBLK], f32r, tag="e", name="e")
                            nc.scalar.activation(e[:], s[:], Exp)
                            pend.append((m, e))
                            # keep PE two steps ahead of ACT before
                            # consuming e with the AV matmul
                            if len(pend) > 2:
                                mm, ee = pend.pop(0)
                                nc.tensor.matmul(
                                    av[:], vx[mm][:, 65 * h:65 * (h + 1)],
                                    ee[:], start=(mm == 0),
                                    stop=(mm == MC - 1))
                        for mm, ee in pend:
                            nc.tensor.matmul(
                                av[:], vx[mm][:, 65 * h:65 * (h + 1)],
                                ee[:], start=(mm == 0), stop=(mm == MC - 1))
                        rec = psm.tile([1, NBLK], f32r, tag="rec", name="rec")
                        with nc.allow_low_precision(reason="f32r is 4-byte"):
                            nc.vector.reciprocal(rec[:], av[64:65, :])
                        bc = ps2.tile([64, NBLK], f32, tag="bc", bufs=2,
                                      name="ps_bc")
                        nc.tensor.matmul(bc[:], ones64, rec[:])
                        bcs = psm.tile([64, NBLK], f32, tag="bcs",
                                       name="bcs")
                        nc.vector.tensor_copy(bcs[:], bc[:])
                        nc.vector.tensor_mul(adst[rr, nsl], av[0:64, :],
                                             bcs[:])

            # ---- Phase 3: output projection (partial) ----
            with tc.tile_pool(name="ps3", bufs=2, space="PSUM") as ps3:
                FCH = [(0, 512), (512, 256)]
                for m in range(MC):
                    msl = slice(128 * m, 128 * (m + 1))
                    osb = posb.tile([128, C], f32, tag="osb", name="osb")
                    for fi, (f0, fn) in enumerate(FCH):
                        ps = ps3.tile([128, fn], f32, tag=f"pj{fi}",
                                      name=f"ps_p{fi}")
                        nc.tensor.matmul(ps[:], aa[:, msl],
                                         wpa[:, f0:f0 + fn],
                                         start=True, stop=False)
                        nc.tensor.matmul(ps[:], ab[:, msl],
                                         wpb[:, f0:f0 + fn],
                                         start=False, stop=True)
                        nc.vector.tensor_copy(osb[:, f0:f0 + fn], ps[:])
                    nc.sync.dma_start(out_d[msl, :], osb[:])

    nc.compile()
    return nc


def kernel(x, w_qkv, b_qkv, w_proj, b_proj):
    from concourse.bass_utils import run_bass_kernel_spmd

    global LAST_RESULTS
    if "nc" not in _cache:
        _cache["nc"] = _build()
    nc = _cache["nc"]

    x = np.asarray(x, dtype=np.float32)
    w_qkv = np.asarray(w_qkv, dtype=np.float32)
    b_qkv = np.asarray(b_qkv, dtype=np.float32)
    w_proj = np.asarray(w_proj, dtype=np.float32)
    b_proj = np.asarray(b_proj, dtype=np.float32)

    in_maps = []
    for c in range(NCORES):
        b = c // 4
        h0 = HPC * (c % 4)
        cs = slice(64 * h0, 64 * (h0 + HPC))
        ks = slice(C + 64 * h0, C + 64 * (h0 + HPC))
        vs = slice(2 * C + 64 * h0, 2 * C + 64 * (h0 + HPC))
        in_maps.append({
            "xt": np.ascontiguousarray(x[b].T),
            "wq": np.ascontiguousarray(w_qkv[:, cs]),
            "wk": np.ascontiguousarray(w_qkv[:, ks]),
            "wv": np.ascontiguousarray(w_qkv[:, vs]),
            "bq": np.ascontiguousarray(
                (b_qkv[cs] * 0.125).reshape(J, 1)),
            "wp": np.ascontiguousarray(w_proj[cs, :]),
            "ones": np.ones((128, 67), dtype=np.float32),
        })

    res = run_bass_kernel_spmd(nc, in_maps, core_ids=list(range(NCORES)))
    LAST_RESULTS = res

    out = np.zeros((B, N, C), dtype=np.float32)
    for c in range(NCORES):
        out[c // 4] += res.results[c]["out"]
    out += b_proj + b_qkv[2 * C:] @ w_proj
    return out


# revision 7
# speedup vs baseline: 1.1820x; 1.1820x over previous
"""Multi-head attention block on 8 TRN2 NeuronCores.

Problem: x[2,2048,768] -> qkv proj -> 12-head attention -> out proj.
Sharding: 24 (batch, head) pairs across 8 cores; core c handles batch
c//4 and heads 3*(c%4)..3*(c%4)+2. Each core computes its heads'
Q,K,V, attention, and a partial output projection; the host sums the
four per-batch partials and adds the bias terms.

Device math notes:
  - All matmuls run in float32r (full-rate PE, ~tf32 precision).
  - Softmax: exp without max subtraction (logits ~ N(0,1), safe in f32);
    denominators come from a ones column appended to V (row 64 of the
    AV output); exp runs on 1024-wide tiles to amortize the ~260ns
    ScalarE instruction overhead so the TensorE stays the bottleneck
    (keeps the PE HAM clock-gate at full 2.4 GHz).
  - Attention output stays UNNORMALIZED; the output projection is done
    per head and each head's contribution is scaled by 1/denominator
    as a per-partition scalar on the DVE. The denominators are moved to
    token-major layout with 16 tiny PE transposes so the reciprocal
    runs wide on 128 lanes instead of one.
  - k-bias is dropped (softmax is shift invariant along keys);
    v-bias and proj-bias are folded in on the host:
      out += b_proj + b_v @ w_proj   (softmax rows sum to 1).
  - q-bias and the 1/sqrt(D) scale are fused into the PSUM->SBUF copy
    of Q^T on the DVE (tensor_scalar mult+add).
"""

import os
import sys

for _p in ("/opt/trn_rl_repo", "/opt/pypackages"):
    if _p not in sys.path:
        sys.path.append(_p)

import numpy as np

B, N, C = 2, 2048, 768
H, D = 12, 64
HPC = 3                    # heads per core
J = HPC * D                # 192: per-core head-dim rows
NCORES = 8
NBLK = 1024                # query-block width (one exp per [128, NBLK])
NB = N // NBLK             # 2
MC = N // 128              # 16 key chunks
KC = C // 128              # 6 contraction chunks for projections

_cache = {}
LAST_RESULTS = None


def _build():
    import concourse.mybir as mybir
    import concourse.tile as tile
    from concourse import bacc

    f32 = mybir.dt.float32
    f32r = mybir.dt.float32r
    Exp = mybir.ActivationFunctionType.Exp
    mult = mybir.AluOpType.mult
    add = mybir.AluOpType.add

    nc = bacc.Bacc("TRN2", target_bir_lowering=False, debug=False,
                   num_devices=NCORES)

    xt_d = nc.declare_dram_parameter("xt", [C, N], f32r, isOutput=False)
    wq_d = nc.declare_dram_parameter("wq", [C, J], f32r, isOutput=False)
    wk_d = nc.declare_dram_parameter("wk", [C, J], f32r, isOutput=False)
    wv_d = nc.declare_dram_parameter("wv", [C, J], f32r, isOutput=False)
    bq_d = nc.declare_dram_parameter("bq", [J, 1], f32, isOutput=False)
    ones_d = nc.declare_dram_parameter("ones", [128, 3], f32r,
                                       isOutput=False)
    iden_d = nc.declare_dram_parameter("iden", [128, 128], f32,
                                       isOutput=False)
    wp_d = nc.declare_dram_parameter("wp", [J, C], f32r, isOutput=False)
    out_d = nc.declare_dram_parameter("out", [N, C], f32, isOutput=True)

    # J=192 rows live on two SBUF tiles: rows 0:128 and 128:192.
    MCH = [(0, 128), (128, 64)]

    with tile.TileContext(nc) as tc:
        with (
            tc.tile_pool(name="persist", bufs=1) as pp,
            tc.tile_pool(name="etile", bufs=3) as pe,
            tc.tile_pool(name="osb", bufs=2) as posb,
        ):
            xt = [pp.tile([128, N], f32r, tag=f"xt{i}", name=f"xt{i}")
                  for i in range(KC)]
            wq = [pp.tile([128, J], f32r, tag=f"wq{i}", name=f"wq{i}")
                  for i in range(KC)]
            wk = [pp.tile([128, J], f32r, tag=f"wk{i}", name=f"wk{i}")
                  for i in range(KC)]
            wv = [pp.tile([128, J], f32r, tag=f"wv{i}", name=f"wv{i}")
                  for i in range(KC)]
            for i in range(KC):
                sl = slice(128 * i, 128 * (i + 1))
                nc.sync.dma_start(xt[i][:], xt_d[sl, :])
                nc.sync.dma_start(wq[i][:], wq_d[sl, :])
                nc.sync.dma_start(wk[i][:], wk_d[sl, :])
                nc.sync.dma_start(wv[i][:], wv_d[sl, :])
            wpa = pp.tile([128, C], f32r, tag="wpa", name="wpa")
            wpb = pp.tile([64, C], f32r, tag="wpb", name="wpb")
            nc.sync.dma_start(wpa[:], wp_d[0:128, :])
            nc.sync.dma_start(wpb[:], wp_d[128:192, :])
            bqa = pp.tile([128, 1], f32, tag="bqa", name="bqa")
            bqb = pp.tile([64, 1], f32, tag="bqb", name="bqb")
            nc.sync.dma_start(bqa[:], bq_d[0:128, :])
            nc.sync.dma_start(bqb[:], bq_d[128:192, :])
            iden = pp.tile([128, 128], f32, tag="iden", name="iden")
            nc.sync.dma_start(iden[:], iden_d[:, :])

            qa = pp.tile([128, N], f32r, tag="qa", name="qa")
            qb = pp.tile([64, N], f32r, tag="qb", name="qb")
            ka = pp.tile([128, N], f32r, tag="ka", name="ka")
            kb = pp.tile([64, N], f32r, tag="kb", name="kb")
            aa = pp.tile([128, N], f32r, tag="aa", name="aa")
            ab = pp.tile([64, N], f32r, tag="ab", name="ab")
            # V with a ones column per head: [128, 3*65]
            vx = [pp.tile([128, HPC * 65], f32r, tag=f"vx{m}", name=f"vx{m}")
                  for m in range(MC)]
            ones_t = pp.tile([128, 3], f32r, tag="ones_t", name="ones_t")
            nc.sync.dma_start(ones_t[:], ones_d[:, :])
            for m in range(MC):
                on = vx[m].rearrange("p (h e) -> p h e", e=65)[:, :, 64:65]
                nc.vector.tensor_copy(
                    on, ones_t[:, :].rearrange("p (h e) -> p h e", e=1))
            # softmax denominators: one row tile per head (partition-0
            # writes only), then token-major recips
            sums = [pp.tile([1, N], f32, tag=f"sums{h}", name=f"sums{h}")
                    for h in range(HPC)]
            rsb = pp.tile([128, HPC * MC], f32, tag="rsb", name="rsb")

            # ---- Phase 1: Q^T, K^T (d-major) and V (token-major) ----
            with tc.tile_pool(name="ps1", bufs=2, space="PSUM") as ps1:
                for t, (r0, rn) in enumerate(MCH):
                    qdst, kdst = (qa, ka) if t == 0 else (qb, kb)
                    bq_t = bqa if t == 0 else bqb
                    for nb in range(4):
                        nsl = slice(512 * nb, 512 * (nb + 1))
                        ps = ps1.tile([128, 512], f32, tag="qk", name="ps_q")
                        for k in range(KC):
                            nc.tensor.matmul(
                                ps[:rn, :], wq[k][:, r0:r0 + rn],
                                xt[k][:, nsl],
                                start=(k == 0), stop=(k == KC - 1))
                        nc.vector.tensor_scalar(
                            qdst[:rn, nsl], ps[:rn, :], 0.125,
                            bq_t[:rn, :], mult, add)
                        ps = ps1.tile([128, 512], f32, tag="qk", name="ps_k")
                        for k in range(KC):
                            nc.tensor.matmul(
                                ps[:rn, :], wk[k][:, r0:r0 + rn],
                                xt[k][:, nsl],
                                start=(k == 0), stop=(k == KC - 1))
                        nc.vector.tensor_copy(kdst[:rn, nsl], ps[:rn, :])
                for m in range(MC):
                    msl = slice(128 * m, 128 * (m + 1))
                    ps = ps1.tile([128, J], f32, tag="v", name="ps_v")
                    for k in range(KC):
                        nc.tensor.matmul(ps[:], xt[k][:, msl], wv[k][:],
                                         start=(k == 0), stop=(k == KC - 1))
                    vdst = vx[m].rearrange("p (h e) -> p h e", e=65)[:, :, 0:64]
                    nc.vector.tensor_copy(
                        vdst, ps.rearrange("p (h e) -> p h e", e=64))

            # ---- Phase 2: attention per head / query block ----
            with tc.tile_pool(name="ps2", bufs=1, space="PSUM") as ps2:
                for h in range(HPC):
                    if h < 2:
                        qsrc, ksrc, adst, r0 = qa, ka, aa, 64 * h
                    else:
                        qsrc, ksrc, adst, r0 = qb, kb, ab, 0
                    rr = slice(r0, r0 + 64)
                    vsl = slice(65 * h, 65 * (h + 1))
                    for nb in range(NB):
                        nsl = slice(NBLK * nb, NBLK * (nb + 1))
                        halves = [slice(NBLK * nb + 512 * i,
                                        NBLK * nb + 512 * (i + 1))
                                  for i in range(NBLK // 512)]
                        av = ps2.tile([65, NBLK], f32, tag="av", bufs=2,
                                      name="ps_av")
                        pend = []
                        for m in range(MC):
                            msl = slice(128 * m, 128 * (m + 1))
                            s = ps2.tile([128, NBLK], f32, tag="s", bufs=2,
                                         name="ps_s")
                            for i, hf in enumerate(halves):
                                nc.tensor.matmul(
                                    s[:, 512 * i:512 * (i + 1)],
                                    ksrc[rr, msl], qsrc[rr, hf])
                            e = pe.tile([128, NBLK], f32r, tag="e", name="e")
                            nc.scalar.activation(e[:], s[:], Exp)
                            pend.append((m, e))
                            # keep PE ~2 steps ahead of ACT before the AV
                            # matmuls consume e
                            if len(pend) > 2:
                                mm, ee = pend.pop(0)
                                for i in range(NBLK // 512):
                                    nc.tensor.matmul(
                                        av[:, 512 * i:512 * (i + 1)],
                                        vx[mm][:, vsl],
                                        ee[:, 512 * i:512 * (i + 1)],
                                        start=(mm == 0), stop=(mm == MC - 1))
                        for mm, ee in pend:
                            for i in range(NBLK // 512):
                                nc.tensor.matmul(
                                    av[:, 512 * i:512 * (i + 1)],
                                    vx[mm][:, vsl],
                                    ee[:, 512 * i:512 * (i + 1)],
                                    start=(mm == 0), stop=(mm == MC - 1))
                        nc.vector.tensor_copy(adst[rr, nsl], av[0:64, :])
                        nc.vector.tensor_copy(sums[h][:, nsl],
                                              av[64:65, :])

            # ---- Phase 2.5: token-major reciprocal denominators ----
            with tc.tile_pool(name="ps25", bufs=2, space="PSUM") as ps25:
                for m in range(MC):
                    msl = slice(128 * m, 128 * (m + 1))
                    tp = ps25.tile([128, HPC], f32, tag="tp", name="ps_tp")
                    for h in range(HPC):
                        nc.tensor.transpose(tp[:, h:h + 1],
                                            sums[h][:, msl],
                                            iden[0:1, 0:1])
                    nc.vector.reciprocal(rsb[:, HPC * m:HPC * (m + 1)],
                                         tp[:])

            # ---- Phase 3: per-head output projection, scaled ----
            with tc.tile_pool(name="ps3", bufs=1, space="PSUM") as ps3:
                FCH = [(0, 512), (512, 256)]
                for m in range(MC):
                    msl = slice(128 * m, 128 * (m + 1))
                    p_h = []
                    for h in range(HPC):
                        asrc = aa if h < 2 else ab
                        wsrc = wpa if h < 2 else wpb
                        r0 = 64 * h if h < 2 else 0
                        hr = slice(r0, r0 + 64)
                        ps = ps3.tile([128, C], f32, tag=f"pj{h}",
                                      name=f"ps_p{h}")
                        for f0, fn in FCH:
                            nc.tensor.matmul(
                                ps[:, f0:f0 + fn], asrc[hr, msl],
                                wsrc[hr, f0:f0 + fn])
                        p_h.append(ps)
                    o1 = posb.tile([128, C], f32, tag="o1", name="o1")
                    o2 = posb.tile([128, C], f32, tag="o2", name="o2")
                    o3 = posb.tile([128, C], f32, tag="o3", name="o3")
                    r = [rsb[:, HPC * m + h:HPC * m + h + 1]
                         for h in range(HPC)]
                    nc.vector.tensor_scalar(o1[:], p_h[0][:], r[0], None,
                                            mult)
                    nc.vector.scalar_tensor_tensor(o2[:], p_h[1][:], r[1],
                                                   o1[:], mult, add)
                    nc.vector.scalar_tensor_tensor(o3[:], p_h[2][:], r[2],
                                                   o2[:], mult, add)
                    nc.sync.dma_start(out_d[msl, :], o3[:])

    nc.compile()
    return nc


def kernel(x, w_qkv, b_qkv, w_proj, b_proj):
    from concourse.bass_utils import run_bass_kernel_spmd

    global LAST_RESULTS
    if "nc" not in _cache:
        _cache["nc"] = _build()
    nc = _cache["nc"]

    x = np.asarray(x, dtype=np.float32)
    w_qkv = np.asarray(w_qkv, dtype=np.float32)
    b_qkv = np.asarray(b_qkv, dtype=np.float32)
    w_proj = np.asarray(w_proj, dtype=np.float32)
    b_proj = np.asarray(b_proj, dtype=np.float32)

    in_maps = []
    for c in range(NCORES):
        b = c // 4
        h0 = HPC * (c % 4)
        cs = slice(64 * h0, 64 * (h0 + HPC))
        ks = slice(C + 64 * h0, C + 64 * (h0 + HPC))
        vs = slice(2 * C + 64 * h0, 2 * C + 64 * (h0 + HPC))
        in_maps.append({
            "xt": np.ascontiguousarray(x[b].T),
            "wq": np.ascontiguousarray(w_qkv[:, cs]),
            "wk": np.ascontiguousarray(w_qkv[:, ks]),
            "wv": np.ascontiguousarray(w_qkv[:, vs]),
            "bq": np.ascontiguousarray(
                (b_qkv[cs] * 0.125).reshape(J, 1)),
            "ones": np.ones((128, 3), dtype=np.float32),
            "iden": np.eye(128, dtype=np.float32),
            "wp": np.ascontiguousarray(w_proj[cs, :]),
        })

    res = run_bass_kernel_spmd(nc, in_maps, core_ids=list(range(NCORES)))
    LAST_RESULTS = res

    out = np.zeros((B, N, C), dtype=np.float32)
    for c in range(NCORES):
        out[c // 4] += res.results[c]["out"]
    out += b_proj + b_qkv[2 * C:] @ w_proj
    return out


# revision 8
# speedup vs baseline: 1.5829x; 1.3392x over previous
"""Multi-head attention block on 8 TRN2 NeuronCores.

Problem: x[2,2048,768] -> qkv proj -> 12-head attention -> out proj.
Sharding: 24 (batch, head) pairs across 8 cores; core c handles batch
c//4 and heads 3*(c%4)..3*(c%4)+2. Each core computes its heads'
Q,K,V, attention, and a partial output projection; the host sums the
four per-batch partials and adds the bias terms.

Device notes:
  - All matmuls run in float32r (full-rate PE, ~tf32 precision).
  - The PE HAM clock-gate only sustains 2.4 GHz when the contraction
    dim drives all 128 array rows; K=64 matmuls pin the clock at
    1.2 GHz. So Q^T/K^T live in per-head [128, N] tiles whose bottom
    64 rows are zeros (zero rhs rows annihilate the don't-care lhsT
    rows), and the per-head output projection uses host-padded
    w_proj blocks. Same math, full clock.
  - Softmax: exp without max subtraction (logits ~ N(0,1)); the
    denominators come from a ones column appended to V (row 64 of the
    AV output); exp runs on 1024-wide tiles to amortize the ~260ns
    ScalarE overhead so TensorE stays the bottleneck.
  - Attention output stays UNNORMALIZED; each head's projection
    contribution is scaled by 1/denominator as a per-partition scalar
    (denominators go token-major via 48 tiny PE transposes so the
    reciprocal runs on 128 DVE lanes instead of one).
  - k-bias is dropped (softmax shift invariance along keys); v-bias
    and proj-bias fold in on the host: out += b_proj + b_v @ w_proj.
  - q-bias and the 1/sqrt(D) scale fuse into the Q^T PSUM->SBUF copy.
"""

import os
import sys

for _p in ("/opt/trn_rl_repo", "/opt/pypackages"):
    if _p not in sys.path:
        sys.path.append(_p)

import numpy as np

B, N, C = 2, 2048, 768
H, D = 12, 64
HPC = 3                    # heads per core
J = HPC * D                # 192: per-core head-dim rows
NCORES = 8
NBLK = 1024                # query-block width (one exp per [128, NBLK])
NB = N // NBLK             # 2
MC = N // 128              # 16 key chunks
KC = C // 128              # 6 contraction chunks for projections

_cache = {}
LAST_RESULTS = None


def _build():
    import concourse.mybir as mybir
    import concourse.tile as tile
    from concourse import bacc

    f32 = mybir.dt.float32
    f32r = mybir.dt.float32r
    Exp = mybir.ActivationFunctionType.Exp
    Copy = mybir.ActivationFunctionType.Copy
    mult = mybir.AluOpType.mult
    add = mybir.AluOpType.add

    nc = bacc.Bacc("TRN2", target_bir_lowering=False, debug=False,
                   num_devices=NCORES)

    xt_d = nc.declare_dram_parameter("xt", [C, N], f32r, isOutput=False)
    wq_d = nc.declare_dram_parameter("wq", [C, J], f32r, isOutput=False)
    wk_d = nc.declare_dram_parameter("wk", [C, J], f32r, isOutput=False)
    wv_d = nc.declare_dram_parameter("wv", [C, J], f32r, isOutput=False)
    bq_d = nc.declare_dram_parameter("bq", [J, 1], f32, isOutput=False)
    ones_d = nc.declare_dram_parameter("ones", [128, 3], f32r,
                                       isOutput=False)
    iden_d = nc.declare_dram_parameter("iden", [128, 128], f32,
                                       isOutput=False)
    # per-head padded proj weights: 3 blocks of [128, C], bottom 64
    # rows of each block are zero
    wp_d = nc.declare_dram_parameter("wp", [HPC * 128, C], f32r,
                                     isOutput=False)
    out_d = nc.declare_dram_parameter("out", [N, C], f32, isOutput=True)

    with tile.TileContext(nc) as tc:
        with (
            tc.tile_pool(name="persist", bufs=1) as pp,
            tc.tile_pool(name="osb", bufs=2) as posb,
        ):
            wq = [pp.tile([128, J], f32r, tag=f"wq{i}", name=f"wq{i}")
                  for i in range(KC)]
            wk = [pp.tile([128, J], f32r, tag=f"wk{i}", name=f"wk{i}")
                  for i in range(KC)]
            wv = [pp.tile([128, J], f32r, tag=f"wv{i}", name=f"wv{i}")
                  for i in range(KC)]
            for i in range(KC):
                sl = slice(128 * i, 128 * (i + 1))
                nc.sync.dma_start(wq[i][:], wq_d[sl, :])
                nc.sync.dma_start(wk[i][:], wk_d[sl, :])
                nc.sync.dma_start(wv[i][:], wv_d[sl, :])
            wp = [pp.tile([128, C], f32r, tag=f"wp{h}", name=f"wp{h}")
                  for h in range(HPC)]
            for h in range(HPC):
                nc.sync.dma_start(wp[h][:], wp_d[128 * h:128 * (h + 1), :])
            bqt = [pp.tile([64, 1], f32, tag=f"bq{h}", name=f"bq{h}")
                   for h in range(HPC)]
            for h in range(HPC):
                nc.sync.dma_start(bqt[h][:], bq_d[64 * h:64 * (h + 1), :])
            iden = pp.tile([128, 128], f32, tag="iden", name="iden")
            nc.sync.dma_start(iden[:], iden_d[:, :])
            ones_t = pp.tile([128, 3], f32r, tag="ones_t", name="ones_t")
            nc.sync.dma_start(ones_t[:], ones_d[:, :])

            # per-head padded Q^T/K^T: rows 0:64 data, rows 64:128 zero
            qh = [pp.tile([128, N], f32r, tag=f"qh{h}", name=f"qh{h}")
                  for h in range(HPC)]
            kh = [pp.tile([128, N], f32r, tag=f"kh{h}", name=f"kh{h}")
                  for h in range(HPC)]
            # V with a ones column per head: [128, 3*65]
            vx = [pp.tile([128, HPC * 65], f32r, tag=f"vx{m}", name=f"vx{m}")
                  for m in range(MC)]
            for m in range(MC):
                on = vx[m].rearrange("p (h e) -> p h e", e=65)[:, :, 64:65]
                nc.vector.tensor_copy(
                    on, ones_t[:, :].rearrange("p (h e) -> p h e", e=1))
            sums = [pp.tile([1, N], f32, tag=f"sums{h}", name=f"sums{h}")
                    for h in range(HPC)]
            rsb = pp.tile([128, HPC * MC], f32, tag="rsb", name="rsb")

            # ---- Phase 1: Q^T, K^T (d-major, padded) and V ----
            with (
                tc.tile_pool(name="xtp", bufs=1) as pxt,
                tc.tile_pool(name="ps1", bufs=2, space="PSUM") as ps1,
            ):
                xt = [pxt.tile([128, N], f32r, tag=f"xt{i}", name=f"xt{i}")
                      for i in range(KC)]
                for i in range(KC):
                    nc.sync.dma_start(xt[i][:],
                                      xt_d[128 * i:128 * (i + 1), :])
                for h in range(HPC):
                    hc = slice(64 * h, 64 * (h + 1))
                    for nb in range(4):
                        nsl = slice(512 * nb, 512 * (nb + 1))
                        ps = ps1.tile([64, 512], f32, tag="qk", name="ps_q")
                        for k in range(KC):
                            nc.tensor.matmul(
                                ps[:], wq[k][:, hc], xt[k][:, nsl],
                                start=(k == 0), stop=(k == KC - 1))
                        nc.vector.tensor_scalar(
                            qh[h][0:64, nsl], ps[:], 0.125,
                            bqt[h][:], mult, add)
                        ps = ps1.tile([64, 512], f32, tag="qk", name="ps_k")
                        for k in range(KC):
                            nc.tensor.matmul(
                                ps[:], wk[k][:, hc], xt[k][:, nsl],
                                start=(k == 0), stop=(k == KC - 1))
                        nc.vector.tensor_copy(kh[h][0:64, nsl], ps[:])
                    # zero the padding rows (annihilates don't-care
                    # operand rows; also keeps NaNs out)
                    nc.vector.tensor_scalar(
                        qh[h][64:128, :], qh[h][0:64, :], 0.0, None, mult)
                    nc.vector.tensor_scalar(
                        kh[h][64:128, :], kh[h][0:64, :], 0.0, None, mult)
                for m in range(MC):
                    msl = slice(128 * m, 128 * (m + 1))
                    ps = ps1.tile([128, J], f32, tag="v", name="ps_v")
                    for k in range(KC):
                        nc.tensor.matmul(ps[:], xt[k][:, msl], wv[k][:],
                                         start=(k == 0), stop=(k == KC - 1))
                    vdst = vx[m].rearrange("p (h e) -> p h e", e=65)[:, :, 0:64]
                    nc.vector.tensor_copy(
                        vdst, ps.rearrange("p (h e) -> p h e", e=64))

            # attention-phase tiles reuse the x^T address range
            with (
                tc.tile_pool(name="attn", bufs=1) as pat,
                tc.tile_pool(name="etile", bufs=3) as pe,
            ):
                ah = [pat.tile([128, N], f32r, tag=f"ah{h}", name=f"ah{h}")
                      for h in range(HPC)]

                # ---- Phase 2: attention per head / query block ----
                with tc.tile_pool(name="ps2", bufs=1, space="PSUM") as ps2:
                    for h in range(HPC):
                        vsl = slice(65 * h, 65 * (h + 1))
                        for nb in range(NB):
                            nsl = slice(NBLK * nb, NBLK * (nb + 1))
                            av = ps2.tile([65, NBLK], f32, tag="av",
                                          bufs=2, name="ps_av")
                            pend = []
                            for m in range(MC):
                                msl = slice(128 * m, 128 * (m + 1))
                                s = ps2.tile([128, NBLK], f32, tag="s",
                                             bufs=2, name="ps_s")
                                for i in range(NBLK // 512):
                                    nc.tensor.matmul(
                                        s[:, 512 * i:512 * (i + 1)],
                                        kh[h][:, msl],
                                        qh[h][:, NBLK * nb + 512 * i:
                                              NBLK * nb + 512 * (i + 1)])
                                e = pe.tile([128, NBLK], f32r, tag="e",
                                            name="e")
                                nc.scalar.activation(e[:], s[:], Exp)
                                pend.append((m, e))
                                # keep PE ~2 steps ahead of ACT
                                if len(pend) > 2:
                                    mm, ee = pend.pop(0)
                                    for i in range(NBLK // 512):
                                        nc.tensor.matmul(
                                            av[:, 512 * i:512 * (i + 1)],
                                            vx[mm][:, vsl],
                                            ee[:, 512 * i:512 * (i + 1)],
                                            start=(mm == 0),
                                            stop=(mm == MC - 1))
                            for mm, ee in pend:
                                for i in range(NBLK // 512):
                                    nc.tensor.matmul(
                                        av[:, 512 * i:512 * (i + 1)],
                                        vx[mm][:, vsl],
                                        ee[:, 512 * i:512 * (i + 1)],
                                        start=(mm == 0), stop=(mm == MC - 1))
                            nc.vector.tensor_copy(ah[h][0:64, nsl],
                                                  av[0:64, :])
                            nc.vector.tensor_copy(sums[h][:, nsl],
                                                  av[64:65, :])
                        nc.vector.tensor_scalar(
                            ah[h][64:128, :], ah[h][0:64, :], 0.0, None,
                            mult)

                # ---- Phase 2.5: token-major reciprocal denominators ----
                with tc.tile_pool(name="ps25", bufs=2, space="PSUM") as ps25:
                    for m in range(MC):
                        msl = slice(128 * m, 128 * (m + 1))
                        tp = ps25.tile([128, HPC], f32, tag="tp",
                                       name="ps_tp")
                        for h in range(HPC):
                            nc.tensor.transpose(tp[:, h:h + 1],
                                                sums[h][:, msl],
                                                iden[0:1, 0:1])
                        nc.vector.reciprocal(rsb[:, HPC * m:HPC * (m + 1)],
                                             tp[:])

                # ---- Phase 3: per-head output projection, scaled ----
                with tc.tile_pool(name="ps3", bufs=1, space="PSUM") as ps3:
                    FCH = [(0, 512), (512, 256)]
                    for m in range(MC):
                        msl = slice(128 * m, 128 * (m + 1))
                        p_h = []
                        for h in range(HPC):
                            ps = ps3.tile([128, C], f32, tag=f"pj{h}",
                                          name=f"ps_p{h}")
                            for f0, fn in FCH:
                                nc.tensor.matmul(
                                    ps[:, f0:f0 + fn], ah[h][:, msl],
                                    wp[h][:, f0:f0 + fn])
                            p_h.append(ps)
                        r = [rsb[:, HPC * m + h:HPC * m + h + 1]
                             for h in range(HPC)]
                        o1 = posb.tile([128, C], f32, tag="o1", name="o1")
                        o2 = posb.tile([128, C], f32, tag="o2", name="o2")
                        o3 = posb.tile([128, C], f32, tag="o3", name="o3")
                        nc.scalar.activation(o1[:], p_h[0][:], Copy,
                                             scale=r[0])
                        nc.scalar.activation(o2[:], p_h[1][:], Copy,
                                             scale=r[1])
                        nc.vector.scalar_tensor_tensor(
                            o3[:], p_h[2][:], r[2], o1[:], mult, add)
                        nc.vector.tensor_add(o3[:], o3[:], o2[:])
                        nc.sync.dma_start(out_d[msl, :], o3[:])

    nc.compile()
    return nc


def kernel(x, w_qkv, b_qkv, w_proj, b_proj):
    from concourse.bass_utils import run_bass_kernel_spmd

    global LAST_RESULTS
    if "nc" not in _cache:
        _cache["nc"] = _build()
    nc = _cache["nc"]

    x = np.asarray(x, dtype=np.float32)
    w_qkv = np.asarray(w_qkv, dtype=np.float32)
    b_qkv = np.asarray(b_qkv, dtype=np.float32)
    w_proj = np.asarray(w_proj, dtype=np.float32)
    b_proj = np.asarray(b_proj, dtype=np.float32)

    in_maps = []
    for c in range(NCORES):
        b = c // 4
        h0 = HPC * (c % 4)
        cs = slice(64 * h0, 64 * (h0 + HPC))
        ks = slice(C + 64 * h0, C + 64 * (h0 + HPC))
        vs = slice(2 * C + 64 * h0, 2 * C + 64 * (h0 + HPC))
        wp_pad = np.zeros((HPC * 128, C), dtype=np.float32)
        for h in range(HPC):
            wp_pad[128 * h:128 * h + 64] = \
                w_proj[64 * (h0 + h):64 * (h0 + h + 1), :]
        in_maps.append({
            "xt": np.ascontiguousarray(x[b].T),
            "wq": np.ascontiguousarray(w_qkv[:, cs]),
            "wk": np.ascontiguousarray(w_qkv[:, ks]),
            "wv": np.ascontiguousarray(w_qkv[:, vs]),
            "bq": np.ascontiguousarray(
                (b_qkv[cs] * 0.125).reshape(J, 1)),
            "ones": np.ones((128, 3), dtype=np.float32),
            "iden": np.eye(128, dtype=np.float32),
            "wp": wp_pad,
        })

    res = run_bass_kernel_spmd(nc, in_maps, core_ids=list(range(NCORES)))
    LAST_RESULTS = res

    out = np.zeros((B, N, C), dtype=np.float32)
    for c in range(NCORES):
        out[c // 4] += res.results[c]["out"]
    out += b_proj + b_qkv[2 * C:] @ w_proj
    return out
